# revision 13
# baseline (speedup 1.0000x reference)
"""Trainium2 Bass kernel for nn_BaseLocalInference (co-attention block).

reference:
    energy = a_hat @ b_hat.T                       # [La, Lb]
    wave_a = softmax(energy, dim=1) @ b_hat        # [La, D]
    wave_b = softmax(energy, dim=0).T @ a_hat      # [Lb, D]
    m_a = concat(a_hat, wave_a, a_hat-wave_a, a_hat*wave_a)   # [4*La, D]
    m_b = concat(b_hat, wave_b, b_hat-wave_b, b_hat*wave_b)   # [4*Lb, D]

Sharding (8 cores): core i owns a-rows [512i, 512i+512) and b-rows likewise.
Both softmaxes are computed exactly with zero mid-kernel reductions by giving
each core the full "other" matrix:

  phase A (per core): Ea^T = B @ A_i^T            [4096(n) x 512(m)]
      rm[m] = max over n  (partition reduce)      -> exact dim-1 stats
      X = exp(Ea^T - rm)                          (lhsT of wave_a)
      wave_a_i = X.T @ [B | 1] -> [512, 1024(+rowsum)] -> normalize
  phase B: identical with roles of A and B swapped -> wave_b_i.

The transposed operands A_i^T/B_i^T are built on-chip with PE transposes and
AllGathered so each core reads the full A^T/B^T without re-transposing.

Matmuls run in float32r (full PE rate); everything else is fp32.
"""
import os
import sys

sys.path.insert(0, os.path.dirname(os.path.abspath(__file__)))

import numpy as np

import concourse.bass as bass
import concourse.tile as tile
from concourse import mybir
from concourse.bass_utils import run_bass_kernel_spmd
from tile_patch import split_multi_waits

P = 128          # partitions
S = 512          # slab rows per core
L = 4096         # La = Lb
D = 1024         # feature dim
NB = 8           # cores
FD = 512         # matmul free dim
F32 = mybir.dt.float32
F32R = mybir.dt.float32r


def _emit_transpose(nc, tp_psum, nat, T, ident):
    """nat [P, 4, D] (slab natural) -> T [P, 8, S] (slab transposed, [d, m])."""
    for t in range(S // P):          # slab row tile
        for c in range(D // P):      # d chunk
            pt = tp_psum.tile([P, P], F32R, name=f"tp_{t}_{c}", tag="tp")
            nc.tensor.transpose(pt[:], nat[:, t, c * P:(c + 1) * P], ident[:])
            nc.scalar.copy(T[:, c, t * P:(t + 1) * P], pt[:])


def _emit_half(nc, tc, blocksT, localT, nat_dram, own_nat, out_dram,
               ones, ones_row, ident_f, tag):
    """One co-attention half. Writes out_dram [3, S, D] = (wave, own-wave, own*wave)."""
    from contextlib import ExitStack

    with ExitStack() as ctx:
        epool = ctx.enter_context(tc.tile_pool(name=f"E{tag}", bufs=1))
        lhs_pool = ctx.enter_context(tc.tile_pool(name=f"lhs{tag}", bufs=2))
        stats = ctx.enter_context(tc.tile_pool(name=f"stats{tag}", bufs=1))

        # E holds the energy tiles [n(128) x (tile k) x m(512)]; declared f32r
        # because the PE consumes it (all writers must carry the f32r label
        # for the BIR verifier). Non-PE readers view it as f32 via bitcast.
        E = epool.tile([P, 32, FD], F32R, name=f"Egt{tag}")

        # ---- energy: E^T tiles [n(128), m(512)] ----
        with tc.tile_pool(name=f"eps{tag}", bufs=4, space="PSUM") as eps:
            for j in range(NB):
                blk = lhs_pool.tile([P, 8, FD], F32R, name=f"blk{tag}", tag="blk")
                nc.sync.dma_start(
                    blk[:], blocksT[j].rearrange("(c p) m -> p c m", p=P)
                )
                for jj in range(4):
                    ps = eps.tile([P, FD], F32, name=f"eps{tag}", tag="eps")
                    for c in range(8):
                        nc.tensor.matmul(
                            ps[:],
                            blk[:, c, jj * P:(jj + 1) * P],
                            localT[:, c, :],
                            start=(c == 0),
                            stop=(c == 7),
                        )
                    nc.scalar.copy(E[:, j * 4 + jj, :], ps[:])

        # ---- stats: rm[m] = max over n (32 tiles then 128 partitions) ----
        sc = stats.tile([P, 4, FD], F32, name=f"sc{tag}")
        for g in range(4):
            nc.vector.tensor_max(
                sc[:, g], E[:, 8 * g].bitcast(F32), E[:, 8 * g + 1].bitcast(F32)
            )
            for u in range(2, 8):
                nc.vector.tensor_max(sc[:, g], sc[:, g], E[:, 8 * g + u].bitcast(F32))
        nc.vector.tensor_max(sc[:, 0], sc[:, 0], sc[:, 1])
        nc.vector.tensor_max(sc[:, 2], sc[:, 2], sc[:, 3])
        nc.vector.tensor_max(sc[:, 0], sc[:, 0], sc[:, 2])
        # partition reduce via PE transpose + free-dim reduce, then broadcast
        # back across partitions with a K=1 ones-matmul.
        rmrow = stats.tile([1, FD], F32, name=f"rmrow{tag}")
        bc = stats.tile([P, FD], F32, name=f"bc{tag}")
        with tc.tile_pool(name=f"stps{tag}", bufs=2, space="PSUM") as stps:
            for j in range(4):
                tp = stps.tile([P, P], F32, name=f"sttp{tag}", tag="st_tp")
                nc.tensor.transpose(tp[:], sc[:, 0, j * P:(j + 1) * P], ident_f[:])
                rmj = stats.tile([P, 1], F32, name=f"rmj{tag}", tag="rmj", bufs=2)
                nc.vector.reduce_max(rmj[:], tp[:], axis=mybir.AxisListType.X)
                tp2 = stps.tile([1, P], F32, name=f"sttp2{tag}", tag="st_tp2")
                nc.tensor.transpose(tp2[:], rmj[:], ident_f[:])
                nc.scalar.copy(rmrow[0:1, j * P:(j + 1) * P], tp2[:])
            bcps = stps.tile([P, FD], F32, name=f"bcps{tag}", tag="bcps")
            nc.tensor.matmul(bcps[:], ones_row[:], rmrow[:],
                             start=True, stop=True)
            nc.scalar.copy(bc[:], bcps[:])

        # ---- X = exp(E - bc), in place, rounded to fp32r for the PE ----
        for k in range(32):
            nc.vector.tensor_sub(E[:, k], E[:, k].bitcast(F32), bc[:])
            nc.scalar.activation(
                E[:, k], E[:, k].bitcast(F32), mybir.ActivationFunctionType.Exp
            )

        # ---- wave = X.T @ [nat | 1], rowsum in the extra column ----
        wpool = ctx.enter_context(tc.tile_pool(name=f"w{tag}", bufs=1))
        wps = ctx.enter_context(tc.tile_pool(name=f"wps{tag}", bufs=1, space="PSUM"))
        sps = ctx.enter_context(tc.tile_pool(name=f"sps{tag}", bufs=1, space="PSUM"))
        rhs_pool = ctx.enter_context(tc.tile_pool(name=f"rhs{tag}", bufs=4))
        wave = wpool.tile([P, 4, D], F32, name=f"wave{tag}")
        rsr = wpool.tile([P, 4], F32, name=f"rsr{tag}")
        for dp in range(2):
            psw = [wps.tile([P, FD], F32, name=f"wps{tag}{dp}_{mt}", tag=f"wps{mt}")
                   for mt in range(4)]
            pss = [sps.tile([P, 1], F32, name=f"sps{tag}{mt}", tag=f"sps{mt}")
                   for mt in range(4)] if dp == 0 else None
            for k in range(32):
                rhs = rhs_pool.tile([P, FD], F32R, name=f"rhs{tag}", tag="rhs")
                nc.sync.dma_start(
                    rhs[:], nat_dram[k * P:(k + 1) * P, dp * FD:(dp + 1) * FD]
                )
                for mt in range(4):
                    lhsT = E[:, k, mt * P:(mt + 1) * P]
                    nc.tensor.matmul(
                        psw[mt][:], lhsT, rhs[:], start=(k == 0), stop=(k == 31)
                    )
                    if dp == 0:
                        nc.tensor.matmul(
                            pss[mt][:], lhsT.bitcast(F32), ones[:],
                            start=(k == 0), stop=(k == 31)
                        )
            if dp == 0:
                for mt in range(4):
                    rs = wpool.tile([P, 1], F32, name=f"rs{tag}{mt}", tag="rs", bufs=4)
                    nc.scalar.copy(rs[:], pss[mt][:])
                    nc.vector.reciprocal(rsr[:, mt:mt + 1], rs[:])
            for mt in range(4):
                nc.vector.tensor_scalar_mul(
                    wave[:, mt, dp * FD:(dp + 1) * FD], psw[mt][:], rsr[:, mt:mt + 1]
                )

        # ---- outputs: wave, own - wave, own * wave ----
        nc.sync.dma_start(
            out_dram[0].rearrange("(t p) d -> p t d", p=P), wave[:]
        )
        opool = ctx.enter_context(tc.tile_pool(name=f"o{tag}", bufs=1))
        for mt in range(4):
            dtile = opool.tile([P, D], F32, name=f"d{tag}", tag="dif")
            nc.vector.tensor_sub(dtile[:], own_nat[:, mt].bitcast(F32), wave[:, mt])
            nc.sync.dma_start(out_dram[1, mt * P:(mt + 1) * P, :], dtile[:])
            ptile = opool.tile([P, D], F32, name=f"p{tag}", tag="prd")
            nc.vector.tensor_mul(ptile[:], own_nat[:, mt].bitcast(F32), wave[:, mt])
            nc.sync.dma_start(out_dram[2, mt * P:(mt + 1) * P, :], ptile[:])


def build_program():
    from contextlib import ExitStack

    nc = bass.Bass()
    a_full = nc.dram_tensor("a_full", [L, D], F32R, kind="ExternalInput")
    b_full = nc.dram_tensor("b_full", [L, D], F32R, kind="ExternalInput")
    a_slab = nc.dram_tensor("a_slab", [S, D], F32R, kind="ExternalInput")
    b_slab = nc.dram_tensor("b_slab", [S, D], F32R, kind="ExternalInput")
    ident_in = nc.dram_tensor("ident", [P, P], F32R, kind="ExternalInput")
    ma = nc.dram_tensor("ma", [3, S, D], F32, kind="ExternalOutput")
    mb = nc.dram_tensor("mb", [3, S, D], F32, kind="ExternalOutput")

    with tile.TileContext(nc) as tc, ExitStack() as ctx:
        const = ctx.enter_context(tc.tile_pool(name="const", bufs=1))
        ident = const.tile([P, P], F32R, name="ident")
        nc.sync.dma_start(ident[:], ident_in[:])
        ident_f = const.tile([P, P], F32, name="ident_f")
        nc.scalar.copy(ident_f[:], ident[:].bitcast(F32))
        ones = const.tile([P, 1], F32, name="ones")
        nc.vector.memset(ones[:], 1.0)
        ones_row = const.tile([1, P], F32, name="ones_row")
        nc.vector.memset(ones_row[:], 1.0)

        slabs = ctx.enter_context(tc.tile_pool(name="slabs", bufs=1))
        a_nat = slabs.tile([P, 4, D], F32R, name="a_nat")
        b_nat = slabs.tile([P, 4, D], F32R, name="b_nat")
        nc.sync.dma_start(a_nat[:], a_slab.rearrange("(t p) d -> p t d", p=P))
        nc.sync.dma_start(b_nat[:], b_slab.rearrange("(t p) d -> p t d", p=P))
        aT = slabs.tile([P, 8, S], F32R, name="aT")
        bT = slabs.tile([P, 8, S], F32R, name="bT")

        with tc.tile_pool(name="tp_psum", bufs=4, space="PSUM") as tp_psum:
            _emit_transpose(nc, tp_psum, b_nat, bT, ident)
            _emit_transpose(nc, tp_psum, a_nat, aT, ident)

        dram = ctx.enter_context(tc.tile_pool(name="dram", bufs=1, space="DRAM"))
        bt_contrib = dram.tile([D, S], F32R, name="bt_contrib")
        at_contrib = dram.tile([D, S], F32R, name="at_contrib")
        bt_all = dram.tile([NB, D, S], F32R, name="bt_all", addr_space="Shared")
        at_all = dram.tile([NB, D, S], F32R, name="at_all", addr_space="Shared")
        nc.sync.dma_start(bt_contrib.rearrange("(c p) m -> p c m", p=P), bT[:])
        nc.sync.dma_start(at_contrib.rearrange("(c p) m -> p c m", p=P), aT[:])
        rg = [list(range(NB))]
        nc.gpsimd.collective_compute(
            "AllGather", mybir.AluOpType.bypass,
            ins=[bt_contrib[:]], outs=[bt_all[:]], replica_groups=rg,
        )
        nc.gpsimd.collective_compute(
            "AllGather", mybir.AluOpType.bypass,
            ins=[at_contrib[:]], outs=[at_all[:]], replica_groups=rg,
        )

        _emit_half(nc, tc, bt_all, aT, b_full, a_nat, ma, ones, ones_row, ident_f, "A")
        _emit_half(nc, tc, at_all, bT, a_full, b_nat, mb, ones, ones_row, ident_f, "B")

    split_multi_waits(nc)
    return nc


_CACHED = {}


def _get_program():
    if "nc" not in _CACHED:
        _CACHED["nc"] = build_program()
    return _CACHED["nc"]


def kernel(a_hat: np.ndarray, b_hat: np.ndarray):
    a_hat = np.ascontiguousarray(np.asarray(a_hat), dtype=np.float32)
    b_hat = np.ascontiguousarray(np.asarray(b_hat), dtype=np.float32)
    nc = _get_program()
    ident_np = np.eye(P, dtype=np.float32)
    in_maps = []
    for i in range(NB):
        in_maps.append({
            "a_full": a_hat,
            "b_full": b_hat,
            "a_slab": np.ascontiguousarray(a_hat[i * S:(i + 1) * S]),
            "b_slab": np.ascontiguousarray(b_hat[i * S:(i + 1) * S]),
            "ident": ident_np,
        })
    res = run_bass_kernel_spmd(nc, in_maps, list(range(NB)))
    wave_a = np.concatenate([res.results[i]["ma"][0] for i in range(NB)], axis=0)
    diff_a = np.concatenate([res.results[i]["ma"][1] for i in range(NB)], axis=0)
    prod_a = np.concatenate([res.results[i]["ma"][2] for i in range(NB)], axis=0)
    wave_b = np.concatenate([res.results[i]["mb"][0] for i in range(NB)], axis=0)
    diff_b = np.concatenate([res.results[i]["mb"][1] for i in range(NB)], axis=0)
    prod_b = np.concatenate([res.results[i]["mb"][2] for i in range(NB)], axis=0)
    m_a = np.concatenate([a_hat, wave_a, diff_a, prod_a], axis=0)
    m_b = np.concatenate([b_hat, wave_b, diff_b, prod_b], axis=0)
    return (m_a, m_b)
